# revision 1
# baseline (speedup 1.0000x reference)
"""Trainium2 Bass kernel: 2-carried scan (cumsum + cumprod) over T for [T, D] fp32.

out1 = cumsum(x, axis=0); out2 = cumprod(2x, axis=0); carries = last rows.
Sharded over 8 NeuronCores along the feature dim D (embarrassingly parallel).

Per-core pipeline (shard W = D/8 = 512 features):
  DMA natural [128T, 2048] tiles -> PE transpose via matmul with 2*I (exact x2)
  -> PSUM [128 feat, 512 T] -> DVE tensor_tensor_scan (sequential fp32 cumsum /
  cumprod, chained across chunks via initial=prev[:, -1:]) -> PE transpose-back
  (0.5*I matmul for cumsum to unscale exactly; pure transpose-mode for cumprod
  so +-inf pass through unmangled) -> ACT copies PSUM->SBUF staging -> DMA out.

The 2-scaling keeps the cumprod input (2x) exact; the cumsum chain runs
2-scaled and is exactly halved on the way out (power-of-2 scaling commutes
with fp32 rounding), so both outputs match sequential fp32 semantics.
"""

import numpy as np

T = 8192
D = 4096
NCORES = 8
W = D // NCORES          # 512 features per core
G = W // 128             # 4 partition groups per core
CHUNK = 512              # T elements per scan chunk (one PSUM bank)
NCHUNK = T // CHUNK      # 16
SUB = CHUNK // 128       # 4 T-subblocks of 128 per chunk

_CACHE = {}


def _build():
    import concourse.bacc as bacc
    import concourse.mybir as mybir
    import concourse.tile as tile

    F32 = mybir.dt.float32
    op_add = mybir.AluOpType.add
    op_mul = mybir.AluOpType.mult
    op_byp = mybir.AluOpType.bypass

    nc = bacc.Bacc("TRN2", debug=False, num_devices=NCORES)
    x = nc.dram_tensor("x", [T, W], F32, kind="ExternalInput")
    i2 = nc.dram_tensor("ident2", [128, 128], F32, kind="ExternalInput")
    ih = nc.dram_tensor("identh", [128, 128], F32, kind="ExternalInput")
    i1 = nc.dram_tensor("ident1", [128, 128], F32, kind="ExternalInput")
    out1 = nc.dram_tensor("out1", [T, W], F32, kind="ExternalOutput")
    out2 = nc.dram_tensor("out2", [T, W], F32, kind="ExternalOutput")

    with tile.TileContext(nc) as tc:
        with (
            tc.tile_pool(name="const", bufs=1) as constp,
            tc.tile_pool(name="nat", bufs=3) as natp,
            tc.tile_pool(name="scan", bufs=12) as scanp,
            tc.tile_pool(name="stage", bufs=3) as stagep,
            tc.tile_pool(name="pin", bufs=3, space="PSUM") as pinp,
            tc.tile_pool(name="pout", bufs=4, space="PSUM") as poutp,
        ):
            i2_t = constp.tile([128, 128], F32, tag="i2")
            ih_t = constp.tile([128, 128], F32, tag="ih")
            i1_t = constp.tile([128, 128], F32, tag="i1")
            dummy = constp.tile([128, CHUNK], F32, tag="dummy")
            nc.sync.dma_start(i2_t[:], i2.ap())
            nc.sync.dma_start(ih_t[:], ih.ap())
            nc.sync.dma_start(i1_t[:], i1.ap())
            nc.vector.memset(dummy[:], 0.0)

            xa = x.ap()
            o1a = out1.ap()
            o2a = out2.ap()
            prev1 = [None] * G
            prev2 = [None] * G
            for c in range(NCHUNK):
                nat = natp.tile([128, SUB * W], F32, tag="nat")
                xin = xa[c * CHUNK:(c + 1) * CHUNK, :].rearrange(
                    "(j p) f -> p j f", p=128)
                nc.sync.dma_start(nat.rearrange("p (j f) -> p j f", j=SUB), xin)
                st1 = stagep.tile([128, SUB * W], F32, tag="st1")
                st2 = stagep.tile([128, SUB * W], F32, tag="st2")
                for g in range(G):
                    pin = pinp.tile([128, CHUNK], F32, tag="pin")
                    for j in range(SUB):
                        nc.tensor.matmul(
                            pin[:, j * 128:(j + 1) * 128],
                            nat[:, j * W + g * 128: j * W + (g + 1) * 128],
                            i2_t[:], start=True, stop=True)
                    s1 = scanp.tile([128, CHUNK], F32, tag="s1")
                    s2 = scanp.tile([128, CHUNK], F32, tag="s2")
                    init1 = 0.0 if c == 0 else prev1[g][:, CHUNK - 1:CHUNK]
                    init2 = 1.0 if c == 0 else prev2[g][:, CHUNK - 1:CHUNK]
                    nc.vector.tensor_tensor_scan(
                        s1[:], pin[:], dummy[:], init1, op_add, op_byp)
                    nc.vector.tensor_tensor_scan(
                        s2[:], pin[:], dummy[:], init2, op_mul, op_byp)
                    prev1[g], prev2[g] = s1, s2
                    po1 = poutp.tile([128, CHUNK], F32, tag="po")
                    po2 = poutp.tile([128, CHUNK], F32, tag="po")
                    for j in range(SUB):
                        nc.tensor.matmul(
                            po1[:, j * 128:(j + 1) * 128],
                            s1[:, j * 128:(j + 1) * 128],
                            ih_t[:], start=True, stop=True)
                    for j in range(SUB):
                        nc.tensor.transpose(
                            po2[:, j * 128:(j + 1) * 128],
                            s2[:, j * 128:(j + 1) * 128],
                            i1_t[:])
                    fsl = slice(g * 128, (g + 1) * 128)
                    nc.scalar.copy(
                        st1.rearrange("p (j f) -> p j f", j=SUB)[:, :, fsl],
                        po1.rearrange("p (j q) -> p j q", j=SUB))
                    nc.scalar.copy(
                        st2.rearrange("p (j f) -> p j f", j=SUB)[:, :, fsl],
                        po2.rearrange("p (j q) -> p j q", j=SUB))
                o1v = o1a[c * CHUNK:(c + 1) * CHUNK, :].rearrange(
                    "(j p) f -> p j f", p=128)
                o2v = o2a[c * CHUNK:(c + 1) * CHUNK, :].rearrange(
                    "(j p) f -> p j f", p=128)
                nc.sync.dma_start(o1v, st1.rearrange("p (j f) -> p j f", j=SUB))
                nc.sync.dma_start(o2v, st2.rearrange("p (j f) -> p j f", j=SUB))
    nc.compile()
    return nc


def _get_nc():
    if "nc" not in _CACHE:
        _CACHE["nc"] = _build()
    return _CACHE["nc"]


def _in_maps(x):
    eye = np.eye(128, dtype=np.float32)
    i2 = (2.0 * eye).astype(np.float32)
    ih = (0.5 * eye).astype(np.float32)
    shards = [np.ascontiguousarray(x[:, i * W:(i + 1) * W]) for i in range(NCORES)]
    return [{"x": s, "ident2": i2, "identh": ih, "ident1": eye} for s in shards]


def run_sharded(x, **kw):
    """Run the SPMD kernel; returns (out1, out2) full arrays plus the raw
    BassKernelResults (for profiling from test harnesses)."""
    from concourse import bass_utils
    nc = _get_nc()
    res = bass_utils.run_bass_kernel_spmd(
        nc, _in_maps(x), core_ids=list(range(NCORES)), **kw)
    out1 = np.concatenate([r["out1"] for r in res.results], axis=1)
    out2 = np.concatenate([r["out2"] for r in res.results], axis=1)
    return out1, out2, res


def kernel(x):
    x = np.asarray(x, dtype=np.float32)
    assert x.shape == (T, D), x.shape
    out1, out2, _ = run_sharded(x)
    carry1 = out1[-1].copy()
    carry2 = out2[-1].copy()
    return (carry1, carry2, out1, out2)


# revision 11
# speedup vs baseline: 1.3673x; 1.3673x over previous
"""Trainium2 Bass kernel: 2-carried scan (cumsum + cumprod) over T for [T, D] fp32.

out1 = cumsum(x, axis=0); out2 = cumprod(2x, axis=0); carries = last rows.
Sharded over 8 NeuronCores along the feature dim D (embarrassingly parallel).

Per-core pipeline (shard W = D/8 = 512 features):
  DMA natural [128T, 2048] tiles -> PE transpose via matmul with 2*I (exact x2)
  -> PSUM [128 feat, 512 T] -> DVE tensor_tensor_scan (sequential fp32 cumsum /
  cumprod, chained across chunks via initial=prev[:, -1:]) -> PE transpose-back
  (0.5*I matmul for cumsum to unscale exactly; pure transpose-mode for cumprod
  so +-inf pass through unmangled) -> ACT copies PSUM->SBUF staging -> DMA out.

The 2-scaling keeps the cumprod input (2x) exact; the cumsum chain runs
2-scaled and is exactly halved on the way out (power-of-2 scaling commutes
with fp32 rounding), so both outputs match sequential fp32 semantics.
"""

import numpy as np

T = 8192
D = 4096
NCORES = 8
W = D // NCORES          # 512 features per core
G = W // 128             # 4 partition groups per core
CHUNK = 512              # T elements per scan chunk (one PSUM bank)
NCHUNK = T // CHUNK      # 16
SUB = CHUNK // 128       # 4 T-subblocks of 128 per chunk

_CACHE = {}


def _build(dma_only=False, no_pe_out=False, no_copies=False, no_scans=False,
           nat_bufs=4, scan_bufs=12, stage_bufs=4, pin_bufs=3, po_bufs=4,
           split_in=1, split_out=2, chunk=CHUNK):
    import concourse.bacc as bacc
    import concourse.mybir as mybir
    import concourse.tile as tile

    F32 = mybir.dt.float32
    op_add = mybir.AluOpType.add
    op_mul = mybir.AluOpType.mult
    op_byp = mybir.AluOpType.bypass

    nc = bacc.Bacc("TRN2", debug=False, num_devices=NCORES)
    x = nc.dram_tensor("x", [T, W], F32, kind="ExternalInput")
    i2 = nc.dram_tensor("ident2", [128, 128], F32, kind="ExternalInput")
    ih = nc.dram_tensor("identh", [128, 128], F32, kind="ExternalInput")
    i1 = nc.dram_tensor("ident1", [128, 128], F32, kind="ExternalInput")
    out1 = nc.dram_tensor("out1", [T, W], F32, kind="ExternalOutput")
    out2 = nc.dram_tensor("out2", [T, W], F32, kind="ExternalOutput")

    nchunk = T // chunk
    sub = chunk // 128
    with tile.TileContext(nc) as tc:
        with (
            tc.tile_pool(name="const", bufs=1) as constp,
            tc.tile_pool(name="nat", bufs=nat_bufs) as natp,
            tc.tile_pool(name="scan", bufs=scan_bufs) as scanp,
            tc.tile_pool(name="stage", bufs=stage_bufs) as stagep,
            tc.tile_pool(name="pin", bufs=pin_bufs, space="PSUM") as pinp,
            tc.tile_pool(name="pout", bufs=po_bufs, space="PSUM") as poutp,
        ):
            i2_t = constp.tile([128, 128], F32, tag="i2")
            ih_t = constp.tile([128, 128], F32, tag="ih")
            i1_t = constp.tile([128, 128], F32, tag="i1")
            dummy = constp.tile([128, chunk], F32, tag="dummy")
            nc.sync.dma_start(i2_t[:], i2.ap())
            nc.sync.dma_start(ih_t[:], ih.ap())
            nc.sync.dma_start(i1_t[:], i1.ap())
            nc.vector.memset(dummy[:], 0.0)

            xa = x.ap()
            o1a = out1.ap()
            o2a = out2.ap()
            prev1 = [None] * G
            prev2 = [None] * G
            for c in range(nchunk):
                nat = natp.tile([128, sub * W], F32, tag="nat")
                if split_in == 1:
                    xin = xa[c * chunk:(c + 1) * chunk, :].rearrange(
                        "(j p) f -> p j f", p=128)
                    nc.sync.dma_start(
                        nat.rearrange("p (j f) -> p j f", j=sub), xin)
                else:
                    # one transfer per T-subblock (finer pipelining)
                    for j in range(sub):
                        r0 = c * chunk + j * 128
                        nc.sync.dma_start(
                            nat[:, j * W:(j + 1) * W], xa[r0:r0 + 128, :])
                st1 = stagep.tile([128, sub * W], F32, tag="st1")
                st2 = stagep.tile([128, sub * W], F32, tag="st2")
                for g in range(G if not dma_only else 0):
                    pin = pinp.tile([128, chunk], F32, tag="pin")
                    for j in range(sub):
                        nc.tensor.matmul(
                            pin[:, j * 128:(j + 1) * 128],
                            nat[:, j * W + g * 128: j * W + (g + 1) * 128],
                            i2_t[:], start=True, stop=True)
                    s1 = scanp.tile([128, chunk], F32, tag="s1")
                    s2 = scanp.tile([128, chunk], F32, tag="s2")
                    if not no_scans:
                        init1 = 0.0 if c == 0 else prev1[g][:, chunk - 1:chunk]
                        init2 = 1.0 if c == 0 else prev2[g][:, chunk - 1:chunk]
                        nc.vector.tensor_tensor_scan(
                            s1[:], pin[:], dummy[:], init1, op_add, op_byp)
                        nc.vector.tensor_tensor_scan(
                            s2[:], pin[:], dummy[:], init2, op_mul, op_byp)
                        prev1[g], prev2[g] = s1, s2
                    if no_pe_out:
                        continue
                    po1 = poutp.tile([128, chunk], F32, tag="po")
                    po2 = poutp.tile([128, chunk], F32, tag="po")
                    for j in range(sub):
                        nc.tensor.matmul(
                            po1[:, j * 128:(j + 1) * 128],
                            s1[:, j * 128:(j + 1) * 128],
                            ih_t[:], start=True, stop=True)
                    for j in range(sub):
                        nc.tensor.transpose(
                            po2[:, j * 128:(j + 1) * 128],
                            s2[:, j * 128:(j + 1) * 128],
                            i1_t[:])
                    if no_copies:
                        continue
                    fsl = slice(g * 128, (g + 1) * 128)
                    nc.scalar.copy(
                        st1.rearrange("p (j f) -> p j f", j=sub)[:, :, fsl],
                        po1.rearrange("p (j q) -> p j q", j=sub))
                    nc.scalar.copy(
                        st2.rearrange("p (j f) -> p j f", j=sub)[:, :, fsl],
                        po2.rearrange("p (j q) -> p j q", j=sub))
                rows = slice(c * chunk, (c + 1) * chunk)
                fw = W // split_out
                for h in range(split_out):
                    cols = slice(h * fw, (h + 1) * fw)
                    o1v = o1a[rows, cols].rearrange("(j p) f -> p j f", p=128)
                    o2v = o2a[rows, cols].rearrange("(j p) f -> p j f", p=128)
                    nc.sync.dma_start(
                        o1v, st1.rearrange("p (j f) -> p j f", j=sub)[:, :, cols])
                    nc.sync.dma_start(
                        o2v, st2.rearrange("p (j f) -> p j f", j=sub)[:, :, cols])
    nc.compile()
    return nc


def _get_nc():
    if "nc" not in _CACHE:
        _CACHE["nc"] = _build()
    return _CACHE["nc"]


def _in_maps(x):
    eye = np.eye(128, dtype=np.float32)
    i2 = (2.0 * eye).astype(np.float32)
    ih = (0.5 * eye).astype(np.float32)
    shards = [np.ascontiguousarray(x[:, i * W:(i + 1) * W]) for i in range(NCORES)]
    return [{"x": s, "ident2": i2, "identh": ih, "ident1": eye} for s in shards]


def run_sharded(x, **kw):
    """Run the SPMD kernel; returns (out1, out2) full arrays plus the raw
    BassKernelResults (for profiling from test harnesses)."""
    from concourse import bass_utils
    nc = _get_nc()
    res = bass_utils.run_bass_kernel_spmd(
        nc, _in_maps(x), core_ids=list(range(NCORES)), **kw)
    out1 = np.concatenate([r["out1"] for r in res.results], axis=1)
    out2 = np.concatenate([r["out2"] for r in res.results], axis=1)
    return out1, out2, res


def kernel(x):
    x = np.asarray(x, dtype=np.float32)
    assert x.shape == (T, D), x.shape
    out1, out2, _ = run_sharded(x)
    carry1 = out1[-1].copy()
    carry2 = out2[-1].copy()
    return (carry1, carry2, out1, out2)
